# revision 1
# baseline (speedup 1.0000x reference)
"""DecoderOnlyAFT Trainium2 kernel: build + host prep/unshard.

Sharding: 8 cores = 4 batches x 2 sequence-halves. Core c -> (b=c//2, half=c%2).
Each core owns 512 tokens; buffer has 640 cols = 64 halo | 512 owned | 64 pad.
Activations feature-major [128 d-part, d-chunk, token-col]; k/v/ek/ekv token-major.
Halo h exchanged per layer via 2-rank AllGather pairs.

Engine balance: matmuls (and k/v bias folds) on PE; PSUM evictions, transcendental
and scale/bias work on ACT; SBUF-only elementwise on DVE (bf16 where inputs are
already bf16-noise-limited, enabling DVE 4x mode).
"""
import sys
sys.path.insert(0, '/opt/trn_rl_repo')
import numpy as np
import concourse.bass as bass
import concourse.mybir as mybir
import concourse.tile as tile
from concourse import bacc
from concourse.masks import make_identity

F32 = mybir.dt.float32
BF16 = mybir.dt.bfloat16
I32 = mybir.dt.int32
AF = mybir.ActivationFunctionType
ALU = mybir.AluOpType

L, D, H, V, T, S_WIN = 6, 512, 2048, 32000, 1024, 64
NB = 640            # buffer cols: 64 halo | 512 owned | 64 pad
NOWN = 512
KC = 4              # d chunks of 128
HC = 16             # hidden chunks of 128
UB = 5              # token-major u-blocks of 128
TB = 4              # owned t-blocks of 128
EPS = 1e-5
SCALE = 1.0 / np.sqrt(np.float32(D))
GROUPS = [[0, 1], [2, 3], [4, 5], [6, 7]]


def build(use_cc=True, mm_bf16=True):
    MMDT = BF16 if mm_bf16 else F32
    nc = bacc.Bacc("TRN2", target_bir_lowering=False, debug=False, num_devices=8)

    toke = nc.dram_tensor("toke", [V, D], F32, kind="ExternalInput")
    ids = nc.dram_tensor("ids", [128, UB], I32, kind="ExternalInput")
    pos = nc.dram_tensor("pos", [128, KC, NB], F32, kind="ExternalInput")
    wq_d = nc.dram_tensor("wq", [L, 128, KC, D], MMDT, kind="ExternalInput")
    wk_d = nc.dram_tensor("wk", [L, 128, KC, D], MMDT, kind="ExternalInput")
    wv_d = nc.dram_tensor("wv", [L, 128, KC, D], MMDT, kind="ExternalInput")
    wo_d = nc.dram_tensor("wo", [L, 128, KC, D], MMDT, kind="ExternalInput")
    w1_d = nc.dram_tensor("w1", [L, 128, KC, H], MMDT, kind="ExternalInput")
    w2_d = nc.dram_tensor("w2", [L, 128, HC, D], MMDT, kind="ExternalInput")
    ewt_d = nc.dram_tensor("ewt", [L, 128, TB, 2, 128], MMDT, kind="ExternalInput")
    bkv_d = nc.dram_tensor("bkv", [L, 1, 2, D], MMDT, kind="ExternalInput")
    sv_d = nc.dram_tensor("sv", [L, 128, 5, KC], F32, kind="ExternalInput")  # bq,bo,b2,g,bln
    b1_d = nc.dram_tensor("b1", [L, 128, HC], F32, kind="ExternalInput")
    out_d = nc.dram_tensor("out", [128, KC, NOWN], F32, kind="ExternalOutput")

    with tile.TileContext(nc) as tc:
        with (
            tc.tile_pool(name="const", bufs=1) as cpool,
            tc.tile_pool(name="hpool", bufs=1) as hpool,
            tc.tile_pool(name="wsm", bufs=1) as wsm,      # wq..wo, ewt (per-layer)
            tc.tile_pool(name="wstream", bufs=4) as wstream,  # w1/w2 chunks
            tc.tile_pool(name="act4", bufs=4) as a4,      # [128,4,512] activations
            tc.tile_pool(name="act5", bufs=1) as a5,      # tokt (init only)
            tc.tile_pool(name="act5b", bufs=3) as a5b,    # ek/v/ekv token-major bf16
            tc.tile_pool(name="misc", bufs=2) as misc,
            tc.tile_pool(name="psmm", bufs=4, space="PSUM") as psmm,
            tc.tile_pool(name="psband", bufs=2, space="PSUM") as psband,
            tc.tile_pool(name="psln", bufs=1, space="PSUM") as psln,
            tc.tile_pool(name="dram", bufs=2, space="DRAM") as dpool,
        ):
            ident = cpool.tile([128, 128], F32)
            make_identity(nc, ident[:])
            ones = cpool.tile([128, 1], F32)
            nc.vector.memset(ones[:], 1.0)
            ones1 = cpool.tile([1, 128], MMDT)
            nc.vector.memset(ones1[:], 1.0)
            ones1f = cpool.tile([1, 128], F32)
            nc.vector.memset(ones1f[:], 1.0)
            epst = cpool.tile([1, 1], F32)
            nc.vector.memset(epst[:], EPS)

            idx = cpool.tile([128, UB], I32)
            nc.sync.dma_start(out=idx[:], in_=ids[:])

            pos_t = hpool.tile([128, KC, NB], F32, tag="pos")
            nc.sync.dma_start(out=pos_t[:], in_=pos[:])

            h = hpool.tile([128, KC, NB], F32, tag="h")

            # ---- embedding: gather rows (token-major), transpose to fm, scale, +pos
            tokt = a5.tile([128, UB, D], F32, tag="a5")
            for ub in range(UB):
                nc.gpsimd.indirect_dma_start(
                    out=tokt[:, ub, :], out_offset=None,
                    in_=toke[:],
                    in_offset=bass.IndirectOffsetOnAxis(ap=idx[:, ub:ub + 1], axis=0),
                )
            for ub in range(UB):
                for dc in range(KC):
                    pst = psband.tile([128, 128], F32, tag="band")
                    nc.tensor.transpose(pst[:], tokt[:, ub, 128 * dc:128 * dc + 128], ident[:])
                    nc.scalar.activation(h[:, dc, 128 * ub:128 * ub + 128], pst[:], AF.Copy,
                                         scale=float(SCALE))
            nc.vector.tensor_add(h[:], h[:], pos_t[:])

            OW = slice(64, 64 + NOWN)  # owned cols in buffer

            for l in range(L):
                # ---- per-layer weights
                wq = wsm.tile([128, KC, D], MMDT, tag="wq")
                nc.sync.dma_start(out=wq[:], in_=wq_d[l])
                wk = wsm.tile([128, KC, D], MMDT, tag="wk")
                nc.sync.dma_start(out=wk[:], in_=wk_d[l])
                wv = wsm.tile([128, KC, D], MMDT, tag="wv")
                nc.sync.dma_start(out=wv[:], in_=wv_d[l])
                wo = wsm.tile([128, KC, D], MMDT, tag="wo")
                nc.sync.dma_start(out=wo[:], in_=wo_d[l])
                ewt = wsm.tile([128, TB, 2, 128], MMDT, tag="ewt")
                nc.sync.dma_start(out=ewt[:], in_=ewt_d[l])
                bkv = misc.tile([1, 2, D], MMDT, tag="bkv")
                nc.sync.dma_start(out=bkv[:], in_=bkv_d[l])
                sv = misc.tile([128, 5, KC], F32, tag="sv")
                nc.sync.dma_start(out=sv[:], in_=sv_d[l])
                b1 = misc.tile([128, HC], F32, tag="b1")
                nc.sync.dma_start(out=b1[:], in_=b1_d[l])

                # ---- emb = h + pos; halo cols (0:64) last — they wait on the AllGather
                emb = hpool.tile([128, KC, NB], F32, tag="emb")
                embm = emb
                if mm_bf16:
                    embm = hpool.tile([128, KC, NB], BF16, tag="embm")
                def emb_part(c0, c1):
                    for kc in range(KC):
                        nc.vector.tensor_add(emb[:, kc, c0:c1], h[:, kc, c0:c1],
                                             pos_t[:, kc, c0:c1])
                        if mm_bf16:
                            nc.vector.tensor_copy(embm[:, kc, c0:c1], emb[:, kc, c0:c1])

                sq = a4.tile([128, KC, NOWN], F32, tag="a4")

                def qf(ch):  # sigmoid(q + bq), one column half
                    cs = slice(64 + 256 * ch, 64 + 256 * ch + 256)
                    os = slice(256 * ch, 256 * ch + 256)
                    for j in range(KC):
                        ps = psmm.tile([128, 256], F32, tag="mm", name=f"psq_{l}_{ch}_{j}")
                        for kc in range(KC):
                            nc.tensor.matmul(ps[:], wq[:, kc, 128 * j:128 * j + 128],
                                             embm[:, kc, cs], start=(kc == 0), stop=(kc == KC - 1))
                        nc.scalar.activation(sq[:, j, os], ps[:], AF.Sigmoid, bias=sv[:, 0, j:j + 1])

                ek = a5b.tile([128, UB, D], MMDT, tag="a5b")
                vb = a5b.tile([128, UB, D], MMDT, tag="a5b")
                ekv = a5b.tile([128, UB, D], MMDT, tag="a5b")

                def kvf(ub):  # k/v for one u-block (bias via K=1 matmul); ek, ekv bf16
                    psk = psmm.tile([128, D], F32, tag="mm", name=f"psk_{l}_{ub}")
                    for kc in range(KC):
                        nc.tensor.matmul(psk[:], embm[:, kc, 128 * ub:128 * ub + 128],
                                         wk[:, kc, :], start=(kc == 0), stop=False)
                    nc.tensor.matmul(psk[:], ones1[:], bkv[:, 0, :], start=False, stop=True)
                    nc.scalar.activation(ek[:, ub, :], psk[:], AF.Exp)
                    psv = psmm.tile([128, D], F32, tag="mm", name=f"psv_{l}_{ub}")
                    for kc in range(KC):
                        nc.tensor.matmul(psv[:], embm[:, kc, 128 * ub:128 * ub + 128],
                                         wv[:, kc, :], start=(kc == 0), stop=False)
                    nc.tensor.matmul(psv[:], ones1[:], bkv[:, 1, :], start=False, stop=True)
                    nc.scalar.activation(vb[:, ub, :], psv[:], AF.Copy)
                    nc.vector.tensor_mul(ekv[:, ub, :], ek[:, ub, :], vb[:, ub, :])

                num = a4.tile([128, KC, NOWN], F32, tag="a4")
                rden = a4.tile([128, KC, NOWN], F32, tag="a4")
                ym = num
                if mm_bf16:
                    ym = a4.tile([128, KC, NOWN], BF16, tag="abf")

                def bandf(i):  # AFT num/den for one t-block; y = sig(q)*num/(den+eps)
                    for dc in range(KC):
                        psn = psband.tile([128, 128], F32, tag="band", name=f"psn_{l}_{i}_{dc}")
                        psd = psband.tile([128, 128], F32, tag="band", name=f"psd_{l}_{i}_{dc}")
                        for sb in range(2):
                            nc.tensor.matmul(psn[:], ekv[:, i + sb, 128 * dc:128 * dc + 128],
                                             ewt[:, i, sb, :], start=(sb == 0), stop=(sb == 1))
                        for sb in range(2):
                            nc.tensor.matmul(psd[:], ek[:, i + sb, 128 * dc:128 * dc + 128],
                                             ewt[:, i, sb, :], start=(sb == 0), stop=(sb == 1))
                        sl = (slice(None), dc, slice(128 * i, 128 * i + 128))
                        nc.scalar.activation(num[sl], psn[:], AF.Copy)
                        nc.scalar.activation(rden[sl], psd[:], AF.Copy, bias=1e-9)
                        nc.vector.reciprocal(rden[sl], rden[sl])
                        nc.vector.tensor_mul(num[sl], num[sl], rden[sl])
                        nc.vector.tensor_mul(ym[sl], num[sl], sq[sl])

                # emission follows data readiness: half1-cols first (h from the
                # previous layer's S4(1), done early), then half0, halo last
                emb_part(320, NB)
                qf(1); kvf(3); kvf(4)
                bandf(3)
                emb_part(64, 320)
                qf(0); kvf(1); kvf(2)
                bandf(1); bandf(2)
                emb_part(0, 64)
                kvf(0)
                bandf(0)

                # ---- column-halved pipeline: Wo/LN1/FF/LN2 per 256-col half so
                #      PE work on one half overlaps ACT/DVE norm work on the other
                x1 = a4.tile([128, KC, NOWN], F32, tag="a4")
                attn = a4.tile([128, KC, NOWN], F32, tag="a4")
                attnm = attn
                if mm_bf16:
                    attnm = a4.tile([128, KC, NOWN], BF16, tag="abf")
                f1 = hpool.tile([128, HC, NOWN], BF16 if mm_bf16 else F32, tag="f1")
                x2 = a4.tile([128, KC, NOWN], F32, tag="a4")
                w1cs, w2cs = [], []
                for hc in range(HC):
                    w1c = wstream.tile([128, KC, 128], MMDT, tag="w1c", bufs=16,
                                       name=f"w1c_{l}_{hc}")
                    nc.sync.dma_start(out=w1c[:], in_=w1_d[l][:, :, 128 * hc:128 * hc + 128])
                    w1cs.append(w1c)
                for j in range(KC):
                    w2c = wstream.tile([128, HC, 128], MMDT, tag="w2c", bufs=4,
                                       name=f"w2c_{l}_{j}")
                    nc.sync.dma_start(out=w2c[:], in_=w2_d[l][:, :, 128 * j:128 * j + 128])
                    w2cs.append(w2c)

                def S0(ch):  # Wo + bias + residual -> x1
                    os = slice(256 * ch, 256 * ch + 256)
                    for j in range(KC):
                        ps = psmm.tile([128, 256], F32, tag="mm", name=f"ps0_{l}_{ch}_{j}")
                        for kc in range(KC):
                            nc.tensor.matmul(ps[:], wo[:, kc, 128 * j:128 * j + 128],
                                             ym[:, kc, os], start=(kc == 0), stop=(kc == KC - 1))
                        nc.scalar.activation(x1[:, j, os], ps[:], AF.Identity,
                                             bias=sv[:, 1, j:j + 1])
                        nc.vector.tensor_tensor(x1[:, j, os], x1[:, j, os],
                                                emb[:, j, 64 + 256 * ch:64 + 256 * ch + 256],
                                                op=ALU.add)

                def S1(ch):  # LN1 -> attn, with bf16 twin written on DVE in parallel
                    layer_norm(nc, a4, psln, misc, psmm, ones, ones1f, epst, x1, sv, attn, ch,
                               out_bf=attnm if mm_bf16 else None)

                def S2(ch):  # FF1 -> f1
                    os = slice(256 * ch, 256 * ch + 256)
                    for hc in range(HC):
                        ps = psmm.tile([128, 256], F32, tag="mm", name=f"ps2_{l}_{ch}_{hc}")
                        for kc in range(KC):
                            nc.tensor.matmul(ps[:], w1cs[hc][:, kc, :],
                                             attnm[:, kc, os], start=(kc == 0), stop=(kc == KC - 1))
                        nc.scalar.activation(f1[:, hc, os], ps[:], AF.Relu, bias=b1[:, hc:hc + 1])

                def S3(ch):  # FF2 + bias + residual -> x2
                    os = slice(256 * ch, 256 * ch + 256)
                    for j in range(KC):
                        ps = psmm.tile([128, 256], F32, tag="mm", name=f"ps3_{l}_{ch}_{j}")
                        for hc in range(HC):
                            nc.tensor.matmul(ps[:], w2cs[j][:, hc, :],
                                             f1[:, hc, os], start=(hc == 0), stop=(hc == HC - 1))
                        nc.scalar.activation(x2[:, j, os], ps[:], AF.Identity,
                                             bias=sv[:, 2, j:j + 1])
                        nc.vector.tensor_tensor(x2[:, j, os], x2[:, j, os], attn[:, j, os],
                                                op=ALU.add)

                def S4(ch):  # LN2 writes h owned cols directly
                    layer_norm(nc, a4, psln, misc, psmm, ones, ones1f, epst, x2, sv,
                               h[:, :, OW], ch)

                # software-pipelined emission: PE stage of one half beside
                # ACT/DVE stage of the other (psum tag slots recycle in
                # emission order, so emission order IS the pipeline order)
                def halo_ag():
                    if l < L - 1:
                        if use_cc:
                            ccin = dpool.tile([128, KC, 64], F32, tag="ccin")
                            ccout = dpool.tile([256, KC, 64], F32, tag="ccout")
                            nc.sync.dma_start(out=ccin[:], in_=h[:, :, 512:576])
                            nc.gpsimd.collective_compute(
                                "AllGather", ALU.bypass,
                                replica_groups=GROUPS,
                                ins=[ccin.opt()], outs=[ccout.opt()],
                            )
                            nc.sync.dma_start(out=h[:, :, 0:64], in_=ccout[0:128])
                        else:
                            nc.sync.dma_start(out=h[:, :, 0:64], in_=h[:, :, 512:576])

                S0(1)
                S0(0); S1(1)
                S2(1); S1(0)
                S3(1); S2(0)
                S4(1)
                halo_ag()
                S3(0)
                S4(0)

            nc.sync.dma_start(out=out_d[:, :, 256:], in_=h[:, :, 320:576])
            nc.sync.dma_start(out=out_d[:, :, 0:256], in_=h[:, :, 64:320])

    nc.compile()
    return nc


def layer_norm(nc, a4, psln, misc, psmm, ones, ones1f, epst, x, sv, out, ch, out_bf=None):
    """out[:, :, half] = (x - mu)/sqrt(var+EPS) * g + b over partition(d) axis.
    Operates on the 256-col half `ch` only (column-local per-token stats)."""
    CW = 256
    os = slice(CW * ch, CW * ch + CW)
    ssum = psln.tile([1, CW], F32, tag="lnsum")
    for kc in range(KC):
        nc.tensor.matmul(ssum[:], ones[:], x[:, kc, os], start=(kc == 0), stop=(kc == KC - 1))
    xsq = a4.tile([128, KC, CW], F32, tag="lnsq")
    nc.scalar.activation(xsq[:], x[:, :, os], AF.Square)
    ssum2 = psln.tile([1, CW], F32, tag="lnsum2")
    for kc in range(KC):
        nc.tensor.matmul(ssum2[:], ones[:], xsq[:, kc, :], start=(kc == 0), stop=(kc == KC - 1))
    st = misc.tile([1, 4, CW], F32, tag="lnst", bufs=2)
    mu, m2, r, mr = (st[:, 0, :], st[:, 1, :], st[:, 2, :], st[:, 3, :])
    nc.vector.tensor_scalar_mul(mu, ssum[:], 1.0 / D)
    nc.vector.tensor_scalar_mul(m2, ssum2[:], 1.0 / D)
    nc.vector.tensor_mul(r, mu, mu)
    nc.vector.tensor_tensor(m2, m2, r, op=ALU.subtract)      # m2 = var
    nc.scalar.activation(r, m2, AF.Sqrt, bias=epst[:1, 0:1])  # r = sqrt(var+eps)
    nc.vector.reciprocal(r, r)                               # r = rstd
    nc.vector.tensor_mul(mr, mu, r)                          # mr = mu*rstd
    psr = psmm.tile([128, CW], F32, tag="mm")
    nc.tensor.matmul(psr[:], ones1f[:], r, start=True, stop=True)
    psm = psmm.tile([128, CW], F32, tag="mm")
    nc.tensor.matmul(psm[:], ones1f[:], mr, start=True, stop=True)
    rbc = misc.tile([128, 2, CW], F32, tag="rbc", bufs=2)
    nc.scalar.activation(rbc[:, 0, :], psr[:], AF.Copy)
    nc.scalar.activation(rbc[:, 1, :], psm[:], AF.Copy)
    tn = a4.tile([128, KC, CW], F32, tag="lnsq")
    nc.vector.tensor_tensor(tn[:], x[:, :, os], rbc[:, 0:1, :].to_broadcast([128, KC, CW]),
                            op=ALU.mult)
    nc.vector.tensor_tensor(tn[:], tn[:], rbc[:, 1:2, :].to_broadcast([128, KC, CW]),
                            op=ALU.subtract)
    for kc in range(KC):
        nc.scalar.activation(out[:, kc, os], tn[:, kc, :], AF.Identity,
                             scale=sv[:, 3, kc:kc + 1], bias=sv[:, 4, kc:kc + 1])
    if out_bf is not None:
        for kc in range(KC):
            nc.vector.tensor_scalar(out_bf[:, kc, os], tn[:, kc, :], sv[:, 3, kc:kc + 1],
                                    sv[:, 4, kc:kc + 1], op0=ALU.mult, op1=ALU.add)


# ------------------------- host side -------------------------

def prep_inputs(inputs, mm_bf16=True):
    """inputs: dict from setup_inputs(). Returns per_core list of input dicts."""
    x = np.asarray(inputs['x']).astype(np.int64)
    toke = np.asarray(inputs['tok_emb'], np.float32)
    pose = np.asarray(inputs['pos_emb'], np.float32)
    wb = np.asarray(inputs['w_bias'], np.float32)

    def fm(w, chunks):  # [d_in, n] -> [128, chunks, n]
        return np.ascontiguousarray(w.reshape(chunks, 128, -1).transpose(1, 0, 2))

    import ml_dtypes
    mdt = ml_dtypes.bfloat16 if mm_bf16 else np.float32
    wq = np.stack([fm(np.asarray(inputs['Wq'][l], np.float32), KC) for l in range(L)])
    wk = np.stack([fm(np.asarray(inputs['Wk'][l], np.float32), KC) for l in range(L)])
    wv = np.stack([fm(np.asarray(inputs['Wv'][l], np.float32), KC) for l in range(L)])
    wo = np.stack([fm(np.asarray(inputs['Wo'][l], np.float32), KC) for l in range(L)])
    w1 = np.stack([fm(np.asarray(inputs['W1'][l], np.float32), KC) for l in range(L)])
    w2 = np.stack([fm(np.asarray(inputs['W2'][l], np.float32), HC) for l in range(L)])

    bkv = np.zeros((L, 1, 2, D), np.float32)
    bkv[:, 0, 0] = np.asarray(inputs['bk'], np.float32)
    bkv[:, 0, 1] = np.asarray(inputs['bv'], np.float32)

    def pv(name):  # per-d vector [L, D] -> [L, 128, KC]
        v = np.asarray(inputs[name], np.float32)
        return v.reshape(L, KC, 128).transpose(0, 2, 1)

    sv = np.stack([pv('bq'), pv('bo'), pv('b2'), pv('ln_g'), pv('ln_b')], axis=2)
    sv = np.ascontiguousarray(sv)  # [L, 128, 5, KC]
    b1 = np.ascontiguousarray(
        np.asarray(inputs['b1'], np.float32).reshape(L, HC, 128).transpose(0, 2, 1))

    # ew per layer (global, fp32, matches reference math)
    t = np.arange(T)
    mask = (t[:, None] >= t[None, :]) & (t[:, None] - t[None, :] < S_WIN)
    NEG = np.float32(-1e30)
    ew_all = []
    for l in range(L):
        wm = np.where(mask, wb[l], NEG).astype(np.float32)
        wm = wm - wm.max(axis=1, keepdims=True)
        ew_all.append(np.exp(wm).astype(np.float32))

    per_core = []
    for c in range(8):
        b, half = c // 2, c % 2
        base = half * 512
        # ids [128, UB]
        ids = np.zeros((128, UB), np.int32)
        for ub in range(UB):
            g = base - 64 + 128 * ub + np.arange(128)
            ok = (g >= 0) & (g < T)
            ids[ok, ub] = x[b, g[ok]].astype(np.int32)
        # pos fm [128, KC, NB]
        posb = np.zeros((NB, D), np.float32)
        g = base - 64 + np.arange(NB)
        ok = (g >= 0) & (g < T)
        posb[ok] = pose[g[ok]] * SCALE
        posb = posb.T.reshape(KC, 128, NB).transpose(1, 0, 2)
        # ewt [L, 128, TB, 2, 128]
        ewt = np.zeros((L, 128, TB, 2, 128), np.float32)
        for l in range(L):
            ew = ew_all[l]
            for i in range(TB):
                gt0 = base + 128 * i
                for s in range(2):
                    gu0 = base - 64 + 128 * (i + s)
                    u0, u1 = max(0, gu0), min(T, gu0 + 128)
                    if u1 <= u0:
                        continue
                    sub = ew[gt0:gt0 + 128, u0:u1]     # [t, u]
                    ewt[l, u0 - gu0:u1 - gu0, i, s, :] = sub.T
        per_core.append(dict(
            toke=toke, ids=ids, pos=np.ascontiguousarray(posb),
            wq=wq.astype(mdt), wk=wk.astype(mdt), wv=wv.astype(mdt),
            wo=wo.astype(mdt), w1=w1.astype(mdt), w2=w2.astype(mdt),
            ewt=np.ascontiguousarray(ewt).astype(mdt), bkv=bkv.astype(mdt),
            sv=sv, b1=b1,
        ))
    return per_core


def unshard(results):
    """results: list of 8 dicts with 'out' [128, KC, NOWN] -> [4, T, D]."""
    full = np.zeros((4, T, D), np.float32)
    for c in range(8):
        b, half = c // 2, c % 2
        o = results[c]['out']  # [128, KC, 512]
        full[b, half * 512:(half + 1) * 512, :] = \
            o.transpose(2, 1, 0).reshape(NOWN, D)
    return full


# ------------------------- public entry -------------------------

_NC_CACHE = {}


def _get_nc():
    if 'nc' not in _NC_CACHE:
        _NC_CACHE['nc'] = build(use_cc=True, mm_bf16=True)
    return _NC_CACHE['nc']


def kernel(**inputs) -> np.ndarray:
    """Full-input, full-output DecoderOnlyAFT forward on 8 NeuronCores."""
    from concourse.bass_utils import run_bass_kernel_spmd
    per_core = prep_inputs(inputs, mm_bf16=True)
    nc = _get_nc()
    res = run_bass_kernel_spmd(nc, per_core, core_ids=list(range(8)))
    return unshard(res.results)

